# revision 14
# baseline (speedup 1.0000x reference)
"""Trainium2 Bass kernel for nn_DistributionalQNetwork (C51 categorical projection).

Strategy (8-core pure data parallel, batch sharded):
  - 4-layer MLP (LN+SiLU) in fp16 on the tensor engine, rows-on-partitions,
    PE transposes for activation re-layout, LN stats via bn_stats,
    normalize+SiLU fused into one scalar-engine activation op.
  - Softmax via reduce_max + Exp-with-accum.
  - C51 projection without any per-lane scatter primitives on the compute
    engines: per-row cumulative sums of the (lower/upper) scatter weights
    along atoms, GPSIMD local_scatter of run-end CDF values into bin space
    (f32 scattered as int16 pairs), segmented max-scan forward fill, then
    adjacent difference of the combined CDF.
"""
import sys

sys.path.insert(0, "/opt/trn_rl_repo")

import numpy as np
import concourse.bass as bass
import concourse.bacc as bacc
import concourse.mybir as mybir
from concourse import tile
from concourse.bass_utils import run_bass_kernel_spmd
from concourse import library_config

F32 = mybir.dt.float32
F16 = mybir.dt.float16
I32 = mybir.dt.int32
I16 = mybir.dt.int16
OP = mybir.AluOpType
AF = mybir.ActivationFunctionType

NC = 8
A = 251          # atoms
AC = 252         # atoms + zero column (scatter dest chunk width)
NOBS = 128
NACT = 32
HID = 512
V_MIN, V_MAX = -10.0, 10.0
INV_DZ = 12.5    # 1/delta_z (exact in fp32)


def build_program(rows_per_core: int, use_silu: bool = True, use_affine=(False, False, False), debug=False, repeats=1, hw_rne: bool = True):
    """Emit the Bass program for one core (SPMD across 8)."""
    assert rows_per_core % 512 == 0
    n_super = rows_per_core // 512
    TPC = rows_per_core // 128  # row-tiles per core (cols of the [128, TPC] r/c maps)

    nc = bacc.Bacc("TRN2", target_bir_lowering=False, debug=False, num_devices=NC)

    def din(name, shape, dt):
        return nc.dram_tensor(name, shape, dt, kind="ExternalInput").ap()

    obs = din("obs", (rows_per_core, NOBS), F16)
    act = din("act", (rows_per_core, NACT), F16)
    c2d = din("c2d", (128, TPC), F32)      # bootstrap*discount, row p*TPC? see layout below
    rr2d = din("rr2d", (128, TPC), F32)    # 12.5*rewards + 125
    w1a0 = din("w1a0", (128, HID), F16)
    w1a1 = din("w1a1", (33, HID), F16)     # act rows + bias row
    w2 = din("w2", (HID, 256), F16)
    w3p = din("w3p", (256, 256), F16)
    w4p = din("w4p", (128, 256), F16)
    b2r = din("b2r", (1, 256), F16)
    b3r = din("b3r", (1, 256), F16)
    b4r = din("b4r", (1, 256), F16)
    eye = din("eye", (128, 128), F16)
    z12 = din("z12", (128, 4 * A), F32)    # tiled 4x: 12.5*q_support
    g32 = din("g32", (128, 4 * A), F16)    # 252*g + 2
    rmA = din("rmA", (128, 4 * A), F16)    # cumsum reset mask (0 at chunk starts)
    rmC = din("rmC", (128, 4 * AC), F32)   # maxscan reset mask
    gb = [din(f"gb{i}", (128, 2 * [HID, 256, 128][i]), F32) for i in range(3)] \
        if any(use_affine) else [None] * 3

    out = nc.dram_tensor("out", (rows_per_core, A), F32, kind="ExternalOutput").ap()
    dbg = {}
    if debug:
        for nm, dt in (("d_b", F32), ("d_l32", F32), ("d_m32", F16), ("d_vlo", I32), ("d_L", F32), ("d_e", F16), ("d_wlo", F16), ("d_frac", F16), ("d_p", F16), ("d_rec", F32)):
            dbg[nm] = nc.dram_tensor(nm, (128, 4 * A), dt, kind="ExternalOutput").ap()

    obs_r = obs.rearrange("(p t) d -> p t d", p=128)
    act_r = act.rearrange("(p t) d -> p t d", p=128)
    out_r = out.rearrange("(p t) a -> p t a", p=128)

    with tile.TileContext(nc) as tc:
        with tc.tile_pool(name="const", bufs=1) as cp, \
             tc.tile_pool(name="work", bufs=2) as wp, \
             tc.tile_pool(name="c51", bufs=1) as gp, \
             tc.tile_pool(name="psH", bufs=2, space="PSUM") as psH, \
             tc.tile_pool(name="psT", bufs=2, space="PSUM") as psT:

            # ---- constants ----
            tw1a0 = cp.tile([128, HID], F16)
            nc.sync.dma_start(out=tw1a0, in_=w1a0)
            tw1a1 = cp.tile([33, HID], F16)
            nc.sync.dma_start(out=tw1a1, in_=w1a1)
            tw2 = cp.tile([128, 4, 256], F16)
            for k in range(4):
                nc.sync.dma_start(out=tw2[:, k, :], in_=w2[128 * k:128 * (k + 1), :])
            tw3 = cp.tile([128, 2, 256], F16)
            for k in range(2):
                nc.sync.dma_start(out=tw3[:, k, :], in_=w3p[128 * k:128 * (k + 1), :])
            tw4 = cp.tile([128, 256], F16)
            nc.sync.dma_start(out=tw4, in_=w4p)
            tb2 = cp.tile([1, 256], F16)
            nc.sync.dma_start(out=tb2, in_=b2r)
            tb3 = cp.tile([1, 256], F16)
            nc.sync.dma_start(out=tb3, in_=b3r)
            tb4 = cp.tile([1, 256], F16)
            nc.sync.dma_start(out=tb4, in_=b4r)
            teye = cp.tile([128, 128], F16)
            nc.sync.dma_start(out=teye, in_=eye)
            tones = cp.tile([1, 128], F16)
            nc.vector.memset(tones, 1.0)
            teps = cp.tile([128, 1], F32)
            nc.vector.memset(teps, 1e-5)
            tz12 = cp.tile([128, 4 * A], F32)
            nc.sync.dma_start(out=tz12, in_=z12)
            tg32 = cp.tile([128, 4 * A], F16)
            nc.sync.dma_start(out=tg32, in_=g32)
            trmA = cp.tile([128, 4 * A], F16)
            nc.sync.dma_start(out=trmA, in_=rmA)
            trmC = cp.tile([128, 4 * AC], F32)
            nc.sync.dma_start(out=trmC, in_=rmC)
            tc2d = cp.tile([128, TPC], F32)
            nc.sync.dma_start(out=tc2d, in_=c2d)
            trr2d = cp.tile([128, TPC], F32)
            nc.sync.dma_start(out=trr2d, in_=rr2d)
            tgb = [None] * 3
            for i in range(3):
                if use_affine[i]:
                    Fw = [HID, 256, 128][i]
                    tgb[i] = cp.tile([128, 2 * Fw], F32)
                    nc.sync.dma_start(out=tgb[i], in_=gb[i])

            layer_w = [
                (None, None),          # L1 handled specially
                (tw2, tb2), (tw3, tb3), (tw4, tb4),
            ]

            for _rep_st in range(repeats * n_super):
                st = _rep_st % n_super
                obs4 = wp.tile([128, 4, NOBS], F16, tag="obs4")
                nc.sync.dma_start(out=obs4, in_=obs_r[:, 4 * st:4 * st + 4, :])
                act4 = wp.tile([128, 4, NACT], F16, tag="act4")
                nc.sync.dma_start(out=act4, in_=act_r[:, 4 * st:4 * st + 4, :])

                e_st = gp.tile([128, 4, A], F16, tag="e_st")
                ssum = wp.tile([128, 4], F32, tag="ssum")
                h4st = psH.tile([128, 4, 256], F32, tag="h4", bufs=1)

                for j in range(4):
                    # ---- L1 ----
                    xT0p = psT.tile([128, 128], F16, tag="tp")
                    nc.tensor.transpose(xT0p, obs4[:, j, :], teye)
                    xT1p = psT.tile([32, 128], F16, tag="tp2")
                    nc.tensor.transpose(xT1p, act4[:, j, :], teye)
                    xT0 = wp.tile([128, 128], F16, tag="xT0")
                    nc.scalar.activation(xT0, xT0p, AF.Copy)
                    xT1 = wp.tile([33, 128], F16, tag="xT1")
                    nc.scalar.activation(xT1[0:32, :], xT1p, AF.Copy)
                    nc.vector.memset(xT1[32:33, :], 1.0)
                    h = psH.tile([128, HID], F32, tag="h")
                    nc.tensor.matmul(h, xT0, tw1a0, start=True, stop=False)
                    nc.tensor.matmul(h, xT1, tw1a1, start=False, stop=True)

                    y = None
                    for li in range(3):
                        # LN + SiLU on h[:, :Fw] -> y (fp16)
                        Fw = [HID, 256, 128][li]
                        hv = h[:, 0:Fw]
                        bn6 = wp.tile([128, 6], F32, tag="bn6")
                        nc.vector.bn_stats(bn6, hv)
                        mv = wp.tile([128, 2], F32, tag="mv")
                        nc.vector.bn_aggr(mv, bn6)
                        sd = wp.tile([128, 1], F32, tag="sd")
                        nc.scalar.activation(sd, mv[:, 1:2], AF.Sqrt, bias=teps, scale=1.0)
                        rstd = wp.tile([128, 1], F32, tag="rstd")
                        nc.vector.reciprocal(rstd, sd)
                        negms = wp.tile([128, 1], F32, tag="negms")
                        nc.vector.scalar_tensor_tensor(
                            negms, mv[:, 0:1], -1.0, rstd, OP.mult, OP.mult)
                        y = wp.tile([128, Fw], F16, tag=f"y{li}")
                        if use_affine[li]:
                            u = wp.tile([128, Fw], F32, tag=f"u{li}")
                            nc.vector.tensor_scalar(u, hv, rstd, negms, OP.mult, OP.add)
                            nc.vector.tensor_tensor(u, u, tgb[li][:, 0:Fw], OP.mult)
                            nc.vector.tensor_tensor(u, u, tgb[li][:, Fw:2 * Fw], OP.add)
                            if use_silu:
                                nc.scalar.activation(y, u, AF.Silu)
                            else:
                                sg = wp.tile([128, Fw], F32, tag=f"sg{li}")
                                nc.scalar.activation(sg, u, AF.Sigmoid)
                                nc.vector.tensor_tensor(y, u, sg, OP.mult)
                        elif use_silu:
                            nc.scalar.activation(y, hv, AF.Silu, bias=negms, scale=rstd)
                        else:
                            u = wp.tile([128, Fw], F32, tag=f"u{li}")
                            nc.vector.tensor_scalar(u, hv, rstd, negms, OP.mult, OP.add)
                            sg = wp.tile([128, Fw], F32, tag=f"sg{li}")
                            nc.scalar.activation(sg, u, AF.Sigmoid)
                            nc.vector.tensor_tensor(y, u, sg, OP.mult)
                        # transpose y -> yT, next matmul
                        nk = Fw // 128
                        yTp = psT.tile([128, nk * 128], F16, tag="tp")
                        for k in range(nk):
                            nc.tensor.transpose(
                                yTp[:, 128 * k:128 * (k + 1)], y[:, 128 * k:128 * (k + 1)], teye)
                        yT = wp.tile([128, nk * 128], F16, tag=f"yT{li}")
                        nc.scalar.activation(yT, yTp, AF.Copy)
                        wt, bt = layer_w[li + 1]
                        if li == 2:
                            h = h4st[:, j, :]
                        else:
                            h = psH.tile([128, 256], F32, tag="h")
                        nc.tensor.matmul(h, tones, bt, start=True, stop=False)
                        for k in range(nk):
                            wk = wt[:, k, :] if wt.ndim == 3 else wt
                            nc.tensor.matmul(
                                h, yT[:, 128 * k:128 * (k + 1)], wk,
                                start=False, stop=(k == nk - 1))

                # softmax: one merged rmax, 4 exp ops
                negmax = wp.tile([128, 4], F32, tag="negmax")
                nc.vector.tensor_reduce(
                    negmax, h4st[:, :, 0:A], mybir.AxisListType.X, OP.max, negate=True)
                for j in range(4):
                    nc.scalar.activation(
                        e_st[:, j, :], h4st[:, j, 0:A], AF.Exp, bias=negmax[:, j:j + 1],
                        scale=1.0, accum_out=ssum[:, j:j + 1])

                # ---- C51 on the supertile ----
                W = 4 * A
                recip = wp.tile([128, 4], F32, tag="recip")
                nc.vector.reciprocal(recip, ssum)

                def bc4(t):  # [128,4] -> [128,4,A] broadcast AP
                    return bass.AP(t.tensor, t.offset, [t.ap[0], [t.ap[1][0], 4], [0, A]])

                c4 = wp.tile([128, 4], F32, tag="c4")
                nc.vector.tensor_copy(c4, tc2d[:, 4 * st:4 * st + 4])
                rr4 = wp.tile([128, 4], F32, tag="rr4")
                nc.vector.tensor_copy(rr4, trr2d[:, 4 * st:4 * st + 4])

                b = gp.tile([128, 4, A], F32, tag="b")
                for g in range(4):
                    nc.gpsimd.tensor_scalar(
                        b[:, g, :], tz12[:, 0:A], c4[:, g:g + 1], rr4[:, g:g + 1],
                        OP.mult, OP.add)
                bf = b.rearrange("p g a -> p (g a)")
                nc.vector.tensor_scalar(bf, bf, 0.0, 250.0, OP.max, OP.min)
                if hw_rne:
                    # HW f32->int convert is round-to-nearest-even: round(b-0.5)=floor(b)
                    # (ties at integer b resolve to either neighbor; both are exact
                    # by continuity of the C51 projection in b)
                    fli = gp.tile([128, W], I32, tag="fli")
                    nc.vector.tensor_scalar(fli, bf, -0.5, 249.4, OP.add, OP.min)
                    lf = None
                else:
                    # rounding-mode-agnostic floor (CoreSim truncates)
                    fli = gp.tile([128, W], I32, tag="fli")
                    nc.vector.tensor_copy(fli, bf)
                    ffl = gp.tile([128, W], F32, tag="ffl")
                    nc.gpsimd.tensor_copy(ffl, fli)
                    g1 = gp.tile([128, W], F32, tag="g1")
                    nc.vector.tensor_tensor(g1, ffl, bf, OP.is_gt)
                    lf = gp.tile([128, W], F32, tag="lf")
                    nc.vector.scalar_tensor_tensor(lf, g1, -1.0, ffl, OP.mult, OP.add)
                    nc.vector.tensor_scalar(lf, lf, 249.0, None, OP.min)
                lf16 = gp.tile([128, W], F16, tag="lf16")
                nc.gpsimd.tensor_copy(lf16, fli if lf is None else lf)
                frac = gp.tile([128, W], F16, tag="frac")
                nc.vector.tensor_tensor(frac, bf, lf16, OP.subtract)
                recip16 = wp.tile([128, 4], F16, tag="recip16")
                nc.vector.tensor_copy(recip16, recip)
                p = gp.tile([128, 4, A], F16, tag="p")
                nc.vector.tensor_tensor(p, e_st, bc4(recip16), OP.mult)
                pf = p.rearrange("p g a -> p (g a)")
                w_hi = gp.tile([128, W], F16, tag="w_hi")
                nc.vector.tensor_tensor(w_hi, pf, frac, OP.mult)
                w_lo = gp.tile([128, W], F16, tag="w_lo")
                nc.vector.tensor_tensor(w_lo, pf, w_hi, OP.subtract)
                L = gp.tile([128, W], F32, tag="L")
                nc.vector.tensor_tensor_scan(L, trmA, w_lo, 0.0, OP.mult, OP.add)
                H = gp.tile([128, W], F32, tag="H")
                nc.vector.tensor_tensor_scan(H, trmA, w_hi, 0.0, OP.mult, OP.add)

                l3 = lf16.rearrange("p (g a) -> p g a", g=4)
                m32 = gp.tile([128, 4, A], F16, tag="m32")
                nc.vector.tensor_tensor(
                    m32[:, :, 0:A - 1], l3[:, :, 1:A], l3[:, :, 0:A - 1], OP.not_equal)
                nc.vector.memset(m32[:, :, A - 1:A], 1.0)
                m32f = m32.rearrange("p g a -> p (g a)")
                s1 = gp.tile([128, W], F16, tag="s1")
                nc.vector.tensor_tensor(s1, lf16, tg32, OP.add)
                t32 = gp.tile([128, W], F16, tag="t32")
                nc.vector.tensor_tensor(t32, s1, m32f, OP.mult)
                vlo = gp.tile([128, W], I32, tag="vlo")
                vlo16 = vlo.bitcast(I16).rearrange("p (w two) -> p w two", two=2)
                nc.gpsimd.tensor_scalar(vlo16[:, :, 0], t32, 2.0, -2.0, OP.mult, OP.add)
                nc.gpsimd.tensor_scalar(vlo16[:, :, 1], t32, 2.0, -1.0, OP.mult, OP.add)
                nc.vector.tensor_tensor(t32, t32, m32f, OP.add)
                vhi = gp.tile([128, W], I32, tag="vhi")
                vhi16 = vhi.bitcast(I16).rearrange("p (w two) -> p w two", two=2)
                nc.gpsimd.tensor_scalar(vhi16[:, :, 0], t32, 2.0, -2.0, OP.mult, OP.add)
                nc.gpsimd.tensor_scalar(vhi16[:, :, 1], t32, 2.0, -1.0, OP.mult, OP.add)

                if debug and st == 0:
                    nc.sync.dma_start(out=dbg["d_b"], in_=bf)
                    nc.sync.dma_start(out=dbg["d_l32"], in_=bf)
                    nc.sync.dma_start(out=dbg["d_m32"], in_=m32f)
                    nc.sync.dma_start(out=dbg["d_vlo"], in_=vlo)
                    nc.sync.dma_start(out=dbg["d_L"], in_=L)
                    nc.sync.dma_start(out=dbg["d_e"], in_=e_st.rearrange("p g a -> p (g a)"))
                    nc.sync.dma_start(out=dbg["d_wlo"], in_=w_lo)
                    nc.sync.dma_start(out=dbg["d_frac"], in_=frac)
                    nc.sync.dma_start(out=dbg["d_p"], in_=p.rearrange("p g a -> p (g a)"))
                    nc.sync.dma_start(out=dbg["d_rec"][:, 0:4], in_=recip)
                dlo = gp.tile([128, 2 * 4 * AC], I16, tag="dlo")
                nc.gpsimd.local_scatter(
                    dlo, L.bitcast(I16), vlo.bitcast(I16),
                    channels=128, num_elems=2 * 4 * AC, num_idxs=2 * W)
                dhi = gp.tile([128, 2 * 4 * AC], I16, tag="dhi")
                nc.gpsimd.local_scatter(
                    dhi, H.bitcast(I16), vhi.bitcast(I16),
                    channels=128, num_elems=2 * 4 * AC, num_idxs=2 * W)

                clo = gp.tile([128, 4 * AC], F32, tag="clo")
                nc.vector.tensor_tensor_scan(
                    clo, trmC, dlo.bitcast(F32), 0.0, OP.mult, OP.max)
                chi = gp.tile([128, 4 * AC], F32, tag="chi")
                nc.vector.tensor_tensor_scan(
                    chi, trmC, dhi.bitcast(F32), 0.0, OP.mult, OP.max)
                nc.vector.tensor_tensor(clo, clo, chi, OP.add)
                c3 = clo.rearrange("p (g a) -> p g a", g=4)
                o_st = gp.tile([128, 4, A], F32, tag="o_st")
                nc.vector.tensor_tensor(
                    o_st, c3[:, :, 1:AC], c3[:, :, 0:A], OP.subtract)
                for j in range(4):
                    nc.sync.dma_start(out=out_r[:, 4 * st + j, :], in_=o_st[:, j, :])
    nc.compile()
    return nc


def prep_host(inputs, rows_per_core):
    """Host-side preprocessing shared across cores; returns (consts, per-core fn)."""
    TPC = rows_per_core // 128
    W1, b1 = inputs["W1"], inputs["b1"]
    consts = {}
    consts["w1a0"] = W1[0:128].astype(np.float16)
    consts["w1a1"] = np.vstack([W1[128:160], b1[None, :]]).astype(np.float16)
    consts["w2"] = inputs["W2"].astype(np.float16)
    w3 = np.zeros((256, 256), np.float32); w3[:, 0:128] = inputs["W3"]
    consts["w3p"] = w3.astype(np.float16)
    w4 = np.zeros((128, 256), np.float32); w4[:, 0:A] = inputs["W4"]
    consts["w4p"] = w4.astype(np.float16)
    consts["b2r"] = inputs["b2"][None, :].astype(np.float16)
    b3 = np.zeros((1, 256), np.float32); b3[0, 0:128] = inputs["b3"]
    consts["b3r"] = b3.astype(np.float16)
    b4 = np.zeros((1, 256), np.float32); b4[0, 0:A] = inputs["b4"]
    consts["b4r"] = b4.astype(np.float16)
    consts["eye"] = np.eye(128, dtype=np.float16)
    z12 = (inputs["q_support"].astype(np.float32) * np.float32(INV_DZ))
    consts["z12"] = np.tile(np.tile(z12, 4)[None, :], (128, 1)).astype(np.float32)
    g = (np.repeat(np.arange(4, dtype=np.int32) * AC, A) + 2).astype(np.int32)
    consts["g32"] = np.tile(g[None, :], (128, 1)).astype(np.float16)
    rma = np.ones(4 * A, np.float32); rma[::A] = 0.0
    consts["rmA"] = np.tile(rma[None, :], (128, 1)).astype(np.float16)
    rmc = np.ones(4 * AC, np.float32); rmc[::AC] = 0.0
    consts["rmC"] = np.tile(rmc[None, :], (128, 1))

    use_affine = []
    for i, (gn, bn) in enumerate((("g1", "be1"), ("g2", "be2"), ("g3", "be3"))):
        gv, bv = inputs[gn], inputs[bn]
        aff = not (np.all(gv == 1.0) and np.all(bv == 0.0))
        use_affine.append(aff)
        if aff:
            Fw = [HID, 256, 128][i]
            consts[f"gb{i}"] = np.tile(
                np.concatenate([gv, bv]).astype(np.float32)[None, :], (128, 1))
    return consts, tuple(use_affine)


_CACHE = {}


def kernel(**inputs) -> np.ndarray:
    B = inputs["obs"].shape[0]
    rows_per_core = B // NC
    consts, use_affine = prep_host(inputs, rows_per_core)
    key = (rows_per_core, use_affine)
    if key not in _CACHE:
        _CACHE[key] = build_program(rows_per_core, use_silu=True, use_affine=use_affine)
    nc = _CACHE[key]

    obs16 = inputs["obs"].astype(np.float16)
    act16 = inputs["actions"].astype(np.float16)
    c_all = (inputs["bootstrap"] * inputs["discount"]).astype(np.float32)
    rr_all = (inputs["rewards"] * np.float32(INV_DZ) + np.float32(125.0)).astype(np.float32)

    TPC = rows_per_core // 128
    in_maps = []
    for k in range(NC):
        s = slice(k * rows_per_core, (k + 1) * rows_per_core)
        m = dict(consts)
        m["obs"] = obs16[s]
        m["act"] = act16[s]
        m["c2d"] = c_all[s].reshape(128, TPC)
        m["rr2d"] = rr_all[s].reshape(128, TPC)
        in_maps.append(m)

    res = run_bass_kernel_spmd(nc, in_maps, core_ids=list(range(NC)))
    out = np.concatenate([res.results[k]["out"] for k in range(NC)], axis=0)
    return out.astype(np.float32)


def timed_run(np_inputs):
    """Run once with NTFF tracing and return HW exec time in ns."""
    B = np_inputs["obs"].shape[0]
    rows_per_core = B // NC
    consts, use_affine = prep_host(np_inputs, rows_per_core)
    key = (rows_per_core, use_affine)
    if key not in _CACHE:
        _CACHE[key] = build_program(rows_per_core, use_silu=True, use_affine=use_affine)
    nc = _CACHE[key]
    obs16 = np_inputs["obs"].astype(np.float16)
    act16 = np_inputs["actions"].astype(np.float16)
    c_all = (np_inputs["bootstrap"] * np_inputs["discount"]).astype(np.float32)
    rr_all = (np_inputs["rewards"] * np.float32(INV_DZ) + np.float32(125.0)).astype(np.float32)
    TPC = rows_per_core // 128
    in_maps = []
    for k in range(NC):
        s = slice(k * rows_per_core, (k + 1) * rows_per_core)
        m = dict(consts)
        m["obs"] = obs16[s]
        m["act"] = act16[s]
        m["c2d"] = c_all[s].reshape(128, TPC)
        m["rr2d"] = rr_all[s].reshape(128, TPC)
        in_maps.append(m)
    res = run_bass_kernel_spmd(nc, in_maps, core_ids=list(range(NC)), trace=True)
    return res.exec_time_ns


if __name__ == "__main__":
    pass


# revision 15
# speedup vs baseline: 1.0037x; 1.0037x over previous
"""Trainium2 Bass kernel for nn_DistributionalQNetwork (C51 categorical projection).

Strategy (8-core pure data parallel, batch sharded):
  - 4-layer MLP (LN+SiLU) in fp16 on the tensor engine, rows-on-partitions,
    PE transposes for activation re-layout, LN stats via bn_stats,
    normalize+SiLU fused into one scalar-engine activation op.
  - Softmax via reduce_max + Exp-with-accum.
  - C51 projection without any per-lane scatter primitives on the compute
    engines: per-row cumulative sums of the (lower/upper) scatter weights
    along atoms, GPSIMD local_scatter of run-end CDF values into bin space
    (f32 scattered as int16 pairs), segmented max-scan forward fill, then
    adjacent difference of the combined CDF.
"""
import sys

sys.path.insert(0, "/opt/trn_rl_repo")

import numpy as np
import concourse.bass as bass
import concourse.bacc as bacc
import concourse.mybir as mybir
from concourse import tile
from concourse.bass_utils import run_bass_kernel_spmd
from concourse import library_config

F32 = mybir.dt.float32
F16 = mybir.dt.float16
I32 = mybir.dt.int32
I16 = mybir.dt.int16
OP = mybir.AluOpType
AF = mybir.ActivationFunctionType

NC = 8
A = 251          # atoms
AC = 252         # atoms + zero column (scatter dest chunk width)
NOBS = 128
NACT = 32
HID = 512
V_MIN, V_MAX = -10.0, 10.0
INV_DZ = 12.5    # 1/delta_z (exact in fp32)


def build_program(rows_per_core: int, use_silu: bool = True, use_affine=(False, False, False), debug=False, repeats=1, hw_rne: bool = True):
    """Emit the Bass program for one core (SPMD across 8)."""
    assert rows_per_core % 512 == 0
    n_super = rows_per_core // 512
    TPC = rows_per_core // 128  # row-tiles per core (cols of the [128, TPC] r/c maps)

    nc = bacc.Bacc("TRN2", target_bir_lowering=False, debug=False, num_devices=NC)

    def din(name, shape, dt):
        return nc.dram_tensor(name, shape, dt, kind="ExternalInput").ap()

    obs = din("obs", (rows_per_core, NOBS), F16)
    act = din("act", (rows_per_core, NACT), F16)
    c2d = din("c2d", (128, TPC), F32)      # bootstrap*discount, row p*TPC? see layout below
    rr2d = din("rr2d", (128, TPC), F32)    # 12.5*rewards + 125
    w1a0 = din("w1a0", (128, HID), F16)
    w1a1 = din("w1a1", (33, HID), F16)     # act rows + bias row
    w2 = din("w2", (HID, 256), F16)
    w3p = din("w3p", (256, 256), F16)
    w4p = din("w4p", (128, 256), F16)
    b2r = din("b2r", (1, 256), F16)
    b3r = din("b3r", (1, 256), F16)
    b4r = din("b4r", (1, 256), F16)
    eye = din("eye", (128, 128), F16)
    z12 = din("z12", (128, 4 * A), F32)    # tiled 4x: 12.5*q_support
    g32 = din("g32", (128, 4 * A), F16)    # 252*g + 2
    rmA = din("rmA", (128, 4 * A), F16)    # cumsum reset mask (0 at chunk starts)
    rmC = din("rmC", (128, 4 * AC), F32)   # maxscan reset mask
    gb = [din(f"gb{i}", (128, 2 * [HID, 256, 128][i]), F32) for i in range(3)] \
        if any(use_affine) else [None] * 3

    out = nc.dram_tensor("out", (rows_per_core, A), F32, kind="ExternalOutput").ap()
    dbg = {}
    if debug:
        for nm, dt in (("d_b", F32), ("d_l32", F32), ("d_m32", F16), ("d_vlo", I32), ("d_L", F32), ("d_e", F16), ("d_wlo", F16), ("d_frac", F16), ("d_p", F16), ("d_rec", F32)):
            dbg[nm] = nc.dram_tensor(nm, (128, 4 * A), dt, kind="ExternalOutput").ap()

    obs_r = obs.rearrange("(p t) d -> p t d", p=128)
    act_r = act.rearrange("(p t) d -> p t d", p=128)
    out_r = out.rearrange("(p t) a -> p t a", p=128)

    with tile.TileContext(nc) as tc:
        with tc.tile_pool(name="const", bufs=1) as cp, \
             tc.tile_pool(name="work", bufs=2) as wp, \
             tc.tile_pool(name="c51", bufs=1) as gp, \
             tc.tile_pool(name="psH", bufs=2, space="PSUM") as psH, \
             tc.tile_pool(name="psT", bufs=2, space="PSUM") as psT:

            # ---- constants ----
            tw1a0 = cp.tile([128, HID], F16)
            nc.sync.dma_start(out=tw1a0, in_=w1a0)
            tw1a1 = cp.tile([33, HID], F16)
            nc.sync.dma_start(out=tw1a1, in_=w1a1)
            tw2 = cp.tile([128, 4, 256], F16)
            for k in range(4):
                nc.sync.dma_start(out=tw2[:, k, :], in_=w2[128 * k:128 * (k + 1), :])
            tw3 = cp.tile([128, 2, 256], F16)
            for k in range(2):
                nc.sync.dma_start(out=tw3[:, k, :], in_=w3p[128 * k:128 * (k + 1), :])
            tw4 = cp.tile([128, 256], F16)
            nc.sync.dma_start(out=tw4, in_=w4p)
            tb2 = cp.tile([1, 256], F16)
            nc.sync.dma_start(out=tb2, in_=b2r)
            tb3 = cp.tile([1, 256], F16)
            nc.sync.dma_start(out=tb3, in_=b3r)
            tb4 = cp.tile([1, 256], F16)
            nc.sync.dma_start(out=tb4, in_=b4r)
            teye = cp.tile([128, 128], F16)
            nc.sync.dma_start(out=teye, in_=eye)
            tones = cp.tile([1, 128], F16)
            nc.vector.memset(tones, 1.0)
            teps = cp.tile([128, 1], F32)
            nc.vector.memset(teps, 1e-5)
            tz12 = cp.tile([128, 4 * A], F32)
            nc.sync.dma_start(out=tz12, in_=z12)
            tg32 = cp.tile([128, 4 * A], F16)
            nc.sync.dma_start(out=tg32, in_=g32)
            trmA = cp.tile([128, 4 * A], F16)
            nc.sync.dma_start(out=trmA, in_=rmA)
            trmC = cp.tile([128, 4 * AC], F32)
            nc.sync.dma_start(out=trmC, in_=rmC)
            tc2d = cp.tile([128, TPC], F32)
            nc.sync.dma_start(out=tc2d, in_=c2d)
            trr2d = cp.tile([128, TPC], F32)
            nc.sync.dma_start(out=trr2d, in_=rr2d)
            tgb = [None] * 3
            for i in range(3):
                if use_affine[i]:
                    Fw = [HID, 256, 128][i]
                    tgb[i] = cp.tile([128, 2 * Fw], F32)
                    nc.sync.dma_start(out=tgb[i], in_=gb[i])

            layer_w = [
                (None, None),          # L1 handled specially
                (tw2, tb2), (tw3, tb3), (tw4, tb4),
            ]

            for _rep_st in range(repeats * n_super):
                st = _rep_st % n_super
                obs4 = wp.tile([128, 4, NOBS], F16, tag="obs4")
                nc.sync.dma_start(out=obs4, in_=obs_r[:, 4 * st:4 * st + 4, :])
                act4 = wp.tile([128, 4, NACT], F16, tag="act4")
                nc.sync.dma_start(out=act4, in_=act_r[:, 4 * st:4 * st + 4, :])

                e_st = gp.tile([128, 4, A], F16, tag="e_st")
                ssum = wp.tile([128, 4], F32, tag="ssum")

                for j in range(4):
                    # ---- L1 ----
                    xT0p = psT.tile([128, 128], F16, tag="tp")
                    nc.tensor.transpose(xT0p, obs4[:, j, :], teye)
                    xT1p = psT.tile([32, 128], F16, tag="tp2")
                    nc.tensor.transpose(xT1p, act4[:, j, :], teye)
                    xT0 = wp.tile([128, 128], F16, tag="xT0")
                    nc.scalar.activation(xT0, xT0p, AF.Copy)
                    xT1 = wp.tile([33, 128], F16, tag="xT1")
                    nc.scalar.activation(xT1[0:32, :], xT1p, AF.Copy)
                    nc.vector.memset(xT1[32:33, :], 1.0)
                    h = psH.tile([128, HID], F32, tag="h")
                    nc.tensor.matmul(h, xT0, tw1a0, start=True, stop=False)
                    nc.tensor.matmul(h, xT1, tw1a1, start=False, stop=True)

                    y = None
                    for li in range(3):
                        # LN + SiLU on h[:, :Fw] -> y (fp16)
                        Fw = [HID, 256, 128][li]
                        hv = h[:, 0:Fw]
                        bn6 = wp.tile([128, 6], F32, tag="bn6")
                        nc.vector.bn_stats(bn6, hv)
                        mv = wp.tile([128, 2], F32, tag="mv")
                        nc.vector.bn_aggr(mv, bn6)
                        sd = wp.tile([128, 1], F32, tag="sd")
                        nc.scalar.activation(sd, mv[:, 1:2], AF.Sqrt, bias=teps, scale=1.0)
                        rstd = wp.tile([128, 1], F32, tag="rstd")
                        nc.vector.reciprocal(rstd, sd)
                        negms = wp.tile([128, 1], F32, tag="negms")
                        nc.vector.scalar_tensor_tensor(
                            negms, mv[:, 0:1], -1.0, rstd, OP.mult, OP.mult)
                        y = wp.tile([128, Fw], F16, tag=f"y{li}")
                        if use_affine[li]:
                            u = wp.tile([128, Fw], F32, tag=f"u{li}")
                            nc.vector.tensor_scalar(u, hv, rstd, negms, OP.mult, OP.add)
                            nc.vector.tensor_tensor(u, u, tgb[li][:, 0:Fw], OP.mult)
                            nc.vector.tensor_tensor(u, u, tgb[li][:, Fw:2 * Fw], OP.add)
                            if use_silu:
                                nc.scalar.activation(y, u, AF.Silu)
                            else:
                                sg = wp.tile([128, Fw], F32, tag=f"sg{li}")
                                nc.scalar.activation(sg, u, AF.Sigmoid)
                                nc.vector.tensor_tensor(y, u, sg, OP.mult)
                        elif use_silu:
                            nc.scalar.activation(y, hv, AF.Silu, bias=negms, scale=rstd)
                        else:
                            u = wp.tile([128, Fw], F32, tag=f"u{li}")
                            nc.vector.tensor_scalar(u, hv, rstd, negms, OP.mult, OP.add)
                            sg = wp.tile([128, Fw], F32, tag=f"sg{li}")
                            nc.scalar.activation(sg, u, AF.Sigmoid)
                            nc.vector.tensor_tensor(y, u, sg, OP.mult)
                        # transpose y -> yT, next matmul
                        nk = Fw // 128
                        yTp = psT.tile([128, nk * 128], F16, tag="tp")
                        for k in range(nk):
                            nc.tensor.transpose(
                                yTp[:, 128 * k:128 * (k + 1)], y[:, 128 * k:128 * (k + 1)], teye)
                        yT = wp.tile([128, nk * 128], F16, tag=f"yT{li}")
                        nc.scalar.activation(yT, yTp, AF.Copy)
                        wt, bt = layer_w[li + 1]
                        h = psH.tile([128, 256], F32, tag="h")
                        nc.tensor.matmul(h, tones, bt, start=True, stop=False)
                        for k in range(nk):
                            wk = wt[:, k, :] if wt.ndim == 3 else wt
                            nc.tensor.matmul(
                                h, yT[:, 128 * k:128 * (k + 1)], wk,
                                start=False, stop=(k == nk - 1))
                    negmax = wp.tile([128, 1], F32, tag="negmax")
                    nc.vector.tensor_reduce(
                        negmax, h[:, 0:A], mybir.AxisListType.X, OP.max, negate=True)
                    nc.scalar.activation(
                        e_st[:, j, :], h[:, 0:A], AF.Exp, bias=negmax, scale=1.0,
                        accum_out=ssum[:, j:j + 1])

                # ---- C51 on the supertile ----
                W = 4 * A
                recip = wp.tile([128, 4], F32, tag="recip")
                nc.vector.reciprocal(recip, ssum)

                def bc4(t):  # [128,4] -> [128,4,A] broadcast AP
                    return bass.AP(t.tensor, t.offset, [t.ap[0], [t.ap[1][0], 4], [0, A]])

                c4 = wp.tile([128, 4], F32, tag="c4")
                nc.vector.tensor_copy(c4, tc2d[:, 4 * st:4 * st + 4])
                rr4 = wp.tile([128, 4], F32, tag="rr4")
                nc.vector.tensor_copy(rr4, trr2d[:, 4 * st:4 * st + 4])

                b = gp.tile([128, 4, A], F32, tag="b")
                nc.vector.tensor_tensor(b, tz12.rearrange("p (g a) -> p g a", g=4), bc4(c4), OP.mult)
                nc.vector.tensor_tensor(b, b, bc4(rr4), OP.add)
                bf = b.rearrange("p g a -> p (g a)")
                nc.gpsimd.tensor_scalar(bf, bf, 0.0, 250.0, OP.max, OP.min)
                if hw_rne:
                    # HW f32->int convert is round-to-nearest-even: round(b-0.5)=floor(b)
                    # (ties at integer b resolve to either neighbor; both are exact
                    # by continuity of the C51 projection in b)
                    fli = gp.tile([128, W], I32, tag="fli")
                    nc.vector.tensor_scalar(fli, bf, -0.5, 249.4, OP.add, OP.min)
                    lf = None
                else:
                    # rounding-mode-agnostic floor (CoreSim truncates)
                    fli = gp.tile([128, W], I32, tag="fli")
                    nc.vector.tensor_copy(fli, bf)
                    ffl = gp.tile([128, W], F32, tag="ffl")
                    nc.gpsimd.tensor_copy(ffl, fli)
                    g1 = gp.tile([128, W], F32, tag="g1")
                    nc.vector.tensor_tensor(g1, ffl, bf, OP.is_gt)
                    lf = gp.tile([128, W], F32, tag="lf")
                    nc.vector.scalar_tensor_tensor(lf, g1, -1.0, ffl, OP.mult, OP.add)
                    nc.vector.tensor_scalar(lf, lf, 249.0, None, OP.min)
                lf16 = gp.tile([128, W], F16, tag="lf16")
                nc.gpsimd.tensor_copy(lf16, fli if lf is None else lf)
                frac = gp.tile([128, W], F16, tag="frac")
                nc.vector.tensor_tensor(frac, bf, lf16, OP.subtract)
                recip16 = wp.tile([128, 4], F16, tag="recip16")
                nc.vector.tensor_copy(recip16, recip)
                p = gp.tile([128, 4, A], F16, tag="p")
                nc.vector.tensor_tensor(p, e_st, bc4(recip16), OP.mult)
                pf = p.rearrange("p g a -> p (g a)")
                w_hi = gp.tile([128, W], F16, tag="w_hi")
                nc.vector.tensor_tensor(w_hi, pf, frac, OP.mult)
                w_lo = gp.tile([128, W], F16, tag="w_lo")
                nc.vector.tensor_tensor(w_lo, pf, w_hi, OP.subtract)
                L = gp.tile([128, W], F32, tag="L")
                nc.vector.tensor_tensor_scan(L, trmA, w_lo, 0.0, OP.mult, OP.add)
                H = gp.tile([128, W], F32, tag="H")
                nc.vector.tensor_tensor_scan(H, trmA, w_hi, 0.0, OP.mult, OP.add)

                l3 = lf16.rearrange("p (g a) -> p g a", g=4)
                m32 = gp.tile([128, 4, A], F16, tag="m32")
                nc.vector.tensor_tensor(
                    m32[:, :, 0:A - 1], l3[:, :, 1:A], l3[:, :, 0:A - 1], OP.not_equal)
                nc.vector.memset(m32[:, :, A - 1:A], 1.0)
                m32f = m32.rearrange("p g a -> p (g a)")
                s1 = gp.tile([128, W], F16, tag="s1")
                nc.vector.tensor_tensor(s1, lf16, tg32, OP.add)
                t32 = gp.tile([128, W], F16, tag="t32")
                nc.vector.tensor_tensor(t32, s1, m32f, OP.mult)
                vlo = gp.tile([128, W], I32, tag="vlo")
                vlo16 = vlo.bitcast(I16).rearrange("p (w two) -> p w two", two=2)
                nc.gpsimd.tensor_scalar(vlo16[:, :, 0], t32, 2.0, -2.0, OP.mult, OP.add)
                nc.gpsimd.tensor_scalar(vlo16[:, :, 1], t32, 2.0, -1.0, OP.mult, OP.add)
                nc.vector.tensor_tensor(t32, t32, m32f, OP.add)
                vhi = gp.tile([128, W], I32, tag="vhi")
                vhi16 = vhi.bitcast(I16).rearrange("p (w two) -> p w two", two=2)
                nc.gpsimd.tensor_scalar(vhi16[:, :, 0], t32, 2.0, -2.0, OP.mult, OP.add)
                nc.gpsimd.tensor_scalar(vhi16[:, :, 1], t32, 2.0, -1.0, OP.mult, OP.add)

                if debug and st == 0:
                    nc.sync.dma_start(out=dbg["d_b"], in_=bf)
                    nc.sync.dma_start(out=dbg["d_l32"], in_=bf)
                    nc.sync.dma_start(out=dbg["d_m32"], in_=m32f)
                    nc.sync.dma_start(out=dbg["d_vlo"], in_=vlo)
                    nc.sync.dma_start(out=dbg["d_L"], in_=L)
                    nc.sync.dma_start(out=dbg["d_e"], in_=e_st.rearrange("p g a -> p (g a)"))
                    nc.sync.dma_start(out=dbg["d_wlo"], in_=w_lo)
                    nc.sync.dma_start(out=dbg["d_frac"], in_=frac)
                    nc.sync.dma_start(out=dbg["d_p"], in_=p.rearrange("p g a -> p (g a)"))
                    nc.sync.dma_start(out=dbg["d_rec"][:, 0:4], in_=recip)
                dlo = gp.tile([128, 2 * 4 * AC], I16, tag="dlo")
                nc.gpsimd.local_scatter(
                    dlo, L.bitcast(I16), vlo.bitcast(I16),
                    channels=128, num_elems=2 * 4 * AC, num_idxs=2 * W)
                dhi = gp.tile([128, 2 * 4 * AC], I16, tag="dhi")
                nc.gpsimd.local_scatter(
                    dhi, H.bitcast(I16), vhi.bitcast(I16),
                    channels=128, num_elems=2 * 4 * AC, num_idxs=2 * W)

                clo = gp.tile([128, 4 * AC], F32, tag="clo")
                nc.vector.tensor_tensor_scan(
                    clo, trmC, dlo.bitcast(F32), 0.0, OP.mult, OP.max)
                chi = gp.tile([128, 4 * AC], F32, tag="chi")
                nc.vector.tensor_tensor_scan(
                    chi, trmC, dhi.bitcast(F32), 0.0, OP.mult, OP.max)
                nc.vector.tensor_tensor(clo, clo, chi, OP.add)
                c3 = clo.rearrange("p (g a) -> p g a", g=4)
                o_st = gp.tile([128, 4, A], F32, tag="o_st")
                nc.vector.tensor_tensor(
                    o_st, c3[:, :, 1:AC], c3[:, :, 0:A], OP.subtract)
                for j in range(4):
                    nc.sync.dma_start(out=out_r[:, 4 * st + j, :], in_=o_st[:, j, :])
    nc.compile()
    return nc


def prep_host(inputs, rows_per_core):
    """Host-side preprocessing shared across cores; returns (consts, per-core fn)."""
    TPC = rows_per_core // 128
    W1, b1 = inputs["W1"], inputs["b1"]
    consts = {}
    consts["w1a0"] = W1[0:128].astype(np.float16)
    consts["w1a1"] = np.vstack([W1[128:160], b1[None, :]]).astype(np.float16)
    consts["w2"] = inputs["W2"].astype(np.float16)
    w3 = np.zeros((256, 256), np.float32); w3[:, 0:128] = inputs["W3"]
    consts["w3p"] = w3.astype(np.float16)
    w4 = np.zeros((128, 256), np.float32); w4[:, 0:A] = inputs["W4"]
    consts["w4p"] = w4.astype(np.float16)
    consts["b2r"] = inputs["b2"][None, :].astype(np.float16)
    b3 = np.zeros((1, 256), np.float32); b3[0, 0:128] = inputs["b3"]
    consts["b3r"] = b3.astype(np.float16)
    b4 = np.zeros((1, 256), np.float32); b4[0, 0:A] = inputs["b4"]
    consts["b4r"] = b4.astype(np.float16)
    consts["eye"] = np.eye(128, dtype=np.float16)
    z12 = (inputs["q_support"].astype(np.float32) * np.float32(INV_DZ))
    consts["z12"] = np.tile(np.tile(z12, 4)[None, :], (128, 1)).astype(np.float32)
    g = (np.repeat(np.arange(4, dtype=np.int32) * AC, A) + 2).astype(np.int32)
    consts["g32"] = np.tile(g[None, :], (128, 1)).astype(np.float16)
    rma = np.ones(4 * A, np.float32); rma[::A] = 0.0
    consts["rmA"] = np.tile(rma[None, :], (128, 1)).astype(np.float16)
    rmc = np.ones(4 * AC, np.float32); rmc[::AC] = 0.0
    consts["rmC"] = np.tile(rmc[None, :], (128, 1))

    use_affine = []
    for i, (gn, bn) in enumerate((("g1", "be1"), ("g2", "be2"), ("g3", "be3"))):
        gv, bv = inputs[gn], inputs[bn]
        aff = not (np.all(gv == 1.0) and np.all(bv == 0.0))
        use_affine.append(aff)
        if aff:
            Fw = [HID, 256, 128][i]
            consts[f"gb{i}"] = np.tile(
                np.concatenate([gv, bv]).astype(np.float32)[None, :], (128, 1))
    return consts, tuple(use_affine)


_CACHE = {}


def kernel(**inputs) -> np.ndarray:
    B = inputs["obs"].shape[0]
    rows_per_core = B // NC
    consts, use_affine = prep_host(inputs, rows_per_core)
    key = (rows_per_core, use_affine)
    if key not in _CACHE:
        _CACHE[key] = build_program(rows_per_core, use_silu=True, use_affine=use_affine)
    nc = _CACHE[key]

    obs16 = inputs["obs"].astype(np.float16)
    act16 = inputs["actions"].astype(np.float16)
    c_all = (inputs["bootstrap"] * inputs["discount"]).astype(np.float32)
    rr_all = (inputs["rewards"] * np.float32(INV_DZ) + np.float32(125.0)).astype(np.float32)

    TPC = rows_per_core // 128
    in_maps = []
    for k in range(NC):
        s = slice(k * rows_per_core, (k + 1) * rows_per_core)
        m = dict(consts)
        m["obs"] = obs16[s]
        m["act"] = act16[s]
        m["c2d"] = c_all[s].reshape(128, TPC)
        m["rr2d"] = rr_all[s].reshape(128, TPC)
        in_maps.append(m)

    res = run_bass_kernel_spmd(nc, in_maps, core_ids=list(range(NC)))
    out = np.concatenate([res.results[k]["out"] for k in range(NC)], axis=0)
    return out.astype(np.float32)


def timed_run(np_inputs):
    """Run once with NTFF tracing and return HW exec time in ns."""
    B = np_inputs["obs"].shape[0]
    rows_per_core = B // NC
    consts, use_affine = prep_host(np_inputs, rows_per_core)
    key = (rows_per_core, use_affine)
    if key not in _CACHE:
        _CACHE[key] = build_program(rows_per_core, use_silu=True, use_affine=use_affine)
    nc = _CACHE[key]
    obs16 = np_inputs["obs"].astype(np.float16)
    act16 = np_inputs["actions"].astype(np.float16)
    c_all = (np_inputs["bootstrap"] * np_inputs["discount"]).astype(np.float32)
    rr_all = (np_inputs["rewards"] * np.float32(INV_DZ) + np.float32(125.0)).astype(np.float32)
    TPC = rows_per_core // 128
    in_maps = []
    for k in range(NC):
        s = slice(k * rows_per_core, (k + 1) * rows_per_core)
        m = dict(consts)
        m["obs"] = obs16[s]
        m["act"] = act16[s]
        m["c2d"] = c_all[s].reshape(128, TPC)
        m["rr2d"] = rr_all[s].reshape(128, TPC)
        in_maps.append(m)
    res = run_bass_kernel_spmd(nc, in_maps, core_ids=list(range(NC)), trace=True)
    return res.exec_time_ns


if __name__ == "__main__":
    pass
